# revision 47
# baseline (speedup 1.0000x reference)
"""Multi-head cross-modal attention + residual + LayerNorm on 8 TRN2 cores.

Reference computation (per batch b):
  Q = query @ Wq.T + bq ; K = key @ Wk.T + bk ; V = value @ Wv.T + bv
  attn = softmax(Q K^T / sqrt(D)) per head
  out  = (attn V) @ Wo.T + bo
  y    = LayerNorm(out + query) * gamma + beta

Sharding: 2-D over (batch=4) x (head-group=2). Core c owns batch c//2 and
heads [8*(c%2), 8*(c%2)+8) i.e. a 512-wide slice of the embedding dim for
Q/K/V/ctx. The out-projection over the 512-slice yields partial sums that a
pairwise ReduceScatter (groups [0,1],[2,3],[4,5],[6,7]) combines; each core
then applies residual+LayerNorm to its 512 rows of the sequence and the host
concatenates the 8 [512,1024] results.

Layout strategy: host pre-transposes activations and weights so every matmul
operand already has its contraction dim on SBUF partitions => zero on-device
transposes. Scores are computed transposed (scoresT[j,i]); a ones-column
appended to V makes the softmax denominator fall out of the ctx matmul as
PSUM row 64. Softmax skips the max-subtraction (scores here are ~N(0,1);
max |score| << 80 so fp32 exp cannot overflow).

Precision: activations/weights stream in bf16 (fp32 accumulate in PSUM);
scores run as float32r (TF32-like, full PE rate at N>=256); the softmax
weights, V, and the reduce-scattered partials are bf16; LayerNorm runs in
fp32. Measured end-to-end max error vs the fp32 reference: ~2.7e-4 of the
output absmax.

Schedule: V/Q projections and the first K o-tile run up front; the remaining
K projection is software-pipelined one o-tile ahead inside the (ACT-bound)
attention head loops. The exp over both query halves is a single ScalarE
instruction per (head, j-tile) to amortize ACT instruction overhead.
"""

import sys

if "/opt/trn_rl_repo" not in sys.path:
    sys.path.insert(0, "/opt/trn_rl_repo")

import ml_dtypes
import numpy as np

import concourse.bass as bass  # noqa: F401  (registers types)
import concourse.mybir as mybir
import concourse.tile as tile
from concourse import bacc
from concourse.bass_utils import run_bass_kernel_spmd

F32 = mybir.dt.float32
F32R = mybir.dt.float32r
BF16 = mybir.dt.bfloat16
AF = mybir.ActivationFunctionType
OP = mybir.AluOpType

B, SQ, SK, E, H, D = 4, 1024, 2048, 1024, 16, 64
N_CORES = 8
OS = 512          # per-core slice of the embedding dim (8 heads x 64)
HL = 8            # local heads per core
ROWS = SQ // 2    # sequence rows each core owns after ReduceScatter
EPS = 1e-5

# module-level knobs used by test.py (harness ignores them)
TRACE = False
LAST_RESULTS = None

_NC_CACHE = None


def _build_nc():
    nc = bacc.Bacc(None, target_bir_lowering=False)

    qT = nc.dram_tensor("qT", [E, SQ], BF16, kind="ExternalInput")
    kT = nc.dram_tensor("kT", [E, SK], BF16, kind="ExternalInput")
    vT = nc.dram_tensor("vT", [E, SK], BF16, kind="ExternalInput")
    wqT = nc.dram_tensor("wqT", [E, OS], BF16, kind="ExternalInput")
    wkT = nc.dram_tensor("wkT", [E, OS], BF16, kind="ExternalInput")
    wvT = nc.dram_tensor("wvT", [E, OS], BF16, kind="ExternalInput")
    woT = nc.dram_tensor("woT", [OS, E], F32, kind="ExternalInput")
    bq4 = nc.dram_tensor("bq4", [4, 128], F32, kind="ExternalInput")
    bk4 = nc.dram_tensor("bk4", [4, 128], F32, kind="ExternalInput")
    resid = nc.dram_tensor("resid", [ROWS, E], F32, kind="ExternalInput")
    vec3 = nc.dram_tensor("vec3", [2, E], F32, kind="ExternalInput")
    ones64 = nc.dram_tensor("ones64", [1, 64], F32, kind="ExternalInput")
    ones_bf = nc.dram_tensor("ones_bf", [1, 64], BF16, kind="ExternalInput")
    out = nc.dram_tensor("out", [ROWS, E], F32, kind="ExternalOutput")

    from contextlib import ExitStack

    with ExitStack() as ctx:
        tc = ctx.enter_context(tile.TileContext(nc))
        constp = ctx.enter_context(tc.tile_pool(name="consts", bufs=1))
        wp = ctx.enter_context(tc.tile_pool(name="wp", bufs=9))
        wop = ctx.enter_context(tc.tile_pool(name="wop", bufs=4))
        actp = ctx.enter_context(tc.tile_pool(name="actp", bufs=8))
        qtp = ctx.enter_context(tc.tile_pool(name="qtp", bufs=4))
        ktp = ctx.enter_context(tc.tile_pool(name="ktp", bufs=4))
        vsb = ctx.enter_context(tc.tile_pool(name="vsb", bufs=16))
        expp = ctx.enter_context(tc.tile_pool(name="expp", bufs=2))
        ctxp = ctx.enter_context(tc.tile_pool(name="ctxp", bufs=4))
        stp = ctx.enter_context(tc.tile_pool(name="stp", bufs=2))
        lnp = ctx.enter_context(tc.tile_pool(name="lnp", bufs=2))
        rbp = ctx.enter_context(tc.tile_pool(name="rbp", bufs=2))
        pp = ctx.enter_context(tc.tile_pool(name="pp", bufs=2, space="PSUM"))
        psc = ctx.enter_context(tc.tile_pool(name="psc", bufs=2, space="PSUM"))
        pc = ctx.enter_context(tc.tile_pool(name="pc", bufs=2, space="PSUM"))
        dramp = ctx.enter_context(tc.tile_pool(name="dramp", bufs=1, space="DRAM"))
        if True:
            # ---------------- small constants (cheap DMAs) ----------------
            eps_t = constp.tile([128, 1], F32)
            nc.vector.memset(eps_t, EPS)
            ones_r = constp.tile([1, 64], F32R)
            nc.sync.dma_start(out=ones_r, in_=ones64[:].bitcast(F32R))
            bq_sb = constp.tile([128, 4], F32)
            bk_sb = constp.tile([128, 4], F32)
            for t in range(4):
                nc.sync.dma_start(
                    out=bq_sb[:, t : t + 1],
                    in_=bq4[t : t + 1, :].rearrange("a b -> b a"),
                )
                nc.sync.dma_start(
                    out=bk_sb[:, t : t + 1],
                    in_=bk4[t : t + 1, :].rearrange("a b -> b a"),
                )

            # ------------ small constants (after the big V loads) -----------
            ones_r = constp.tile([1, 64], F32R)
            nc.sync.dma_start(out=ones_r, in_=ones64[:].bitcast(F32R))
            bq_sb = constp.tile([128, 4], F32)
            bk_sb = constp.tile([128, 4], F32)
            for t in range(4):
                nc.sync.dma_start(
                    out=bq_sb[:, t : t + 1],
                    in_=bq4[t : t + 1, :].rearrange("a b -> b a"),
                )
                nc.sync.dma_start(
                    out=bk_sb[:, t : t + 1],
                    in_=bk4[t : t + 1, :].rearrange("a b -> b a"),
                )

            # ---------------- Q projection: QT[o, i] ----------------
            qt_in = []
            wq_t = []
            for e in range(8):
                w = wp.tile([128, OS], BF16, tag="w", name=f"wq_{e}")
                nc.sync.dma_start(
                    out=w, in_=wqT[e * 128 : (e + 1) * 128, :]
                )
                wq_t.append(w)
                a = actp.tile([128, SQ], BF16, tag="act", name=f"qin_{e}")
                nc.sync.dma_start(
                    out=a, in_=qT[e * 128 : (e + 1) * 128, :]
                )
                qt_in.append(a)
            QTt = [
                qtp.tile([128, SQ], BF16, tag="qt", name=f"QT_{ot}")
                for ot in range(4)
            ]
            for ot in range(4):
                for ih in range(2):
                    p = pp.tile([128, 512], F32, tag="proj", name=f"pq_{ot}_{ih}")
                    for e in range(8):
                        nc.tensor.matmul(
                            p[:],
                            wq_t[e][:, ot * 128 : (ot + 1) * 128],
                            qt_in[e][:, ih * 512 : (ih + 1) * 512],
                            start=(e == 0),
                            stop=(e == 7),
                        )
                    nc.vector.tensor_scalar_add(
                        out=QTt[ot][:, ih * 512 : (ih + 1) * 512],
                        in0=p[:],
                        scalar1=bq_sb[:, ot : ot + 1],
                    )

            # ---------------- K projection: KT[o, j] ----------------
            wk_t = []
            KTt = [
                ktp.tile([128, SK], BF16, tag="kt", name=f"KT_{ot}")
                for ot in range(4)
            ]
            for jh in range(2):
                kt_in = []
                for e in range(8):
                    if jh == 0:
                        w = wp.tile([128, OS], BF16, tag="w", name=f"wk_{e}")
                        nc.sync.dma_start(
                            out=w,
                            in_=wkT[e * 128 : (e + 1) * 128, :],
                        )
                        wk_t.append(w)
                    a = actp.tile([128, 1024], BF16, tag="act", name=f"kin_{jh}_{e}")
                    nc.sync.dma_start(
                        out=a,
                        in_=kT[
                            e * 128 : (e + 1) * 128, jh * 1024 : (jh + 1) * 1024
                        ],
                    )
                    kt_in.append(a)
                for ot in range(4):
                    for jc in range(2):
                        p = pp.tile(
                            [128, 512], F32, tag="proj", name=f"pk_{jh}_{ot}_{jc}"
                        )
                        for e in range(8):
                            nc.tensor.matmul(
                                p[:],
                                wk_t[e][:, ot * 128 : (ot + 1) * 128],
                                kt_in[e][:, jc * 512 : (jc + 1) * 512],
                                start=(e == 0),
                                stop=(e == 7),
                            )
                        off = jh * 1024 + jc * 512
                        nc.vector.tensor_scalar_add(
                            out=KTt[ot][:, off : off + 512],
                            in0=p[:],
                            scalar1=bk_sb[:, ot : ot + 1],
                        )

            # ---------------- V projection: V[j, (h,65)] with ones col ----
            wv_t = []
            v_tiles = [
                vsb.tile([128, HL * 65], BF16, tag="v", name=f"V_{jt}")
                for jt in range(16)
            ]
            for jh in range(2):
                vt_in = []
                for e in range(8):
                    if jh == 0:
                        w = wp.tile([128, OS], BF16, tag="w", name=f"wv_{e}")
                        nc.sync.dma_start(
                            out=w,
                            in_=wvT[e * 128 : (e + 1) * 128, :],
                        )
                        wv_t.append(w)
                    a = actp.tile([128, 1024], BF16, tag="act", name=f"vin_{jh}_{e}")
                    nc.sync.dma_start(
                        out=a,
                        in_=vT[
                            e * 128 : (e + 1) * 128, jh * 1024 : (jh + 1) * 1024
                        ],
                    )
                    vt_in.append(a)
                for jq4 in range(8):
                    jt = jh * 8 + jq4
                    vv = v_tiles[jt].rearrange("p (h c) -> p h c", h=HL)
                    nc.vector.memset(vv[:, :, 64:65], 1.0)
                for jq in range(4):
                    pvs = [
                        pp.tile([128, 512], F32, tag="proj", name=f"pv_{jh}_{jq}_{k}")
                        for k in range(2)
                    ]
                    for e in range(8):
                        for k in range(2):
                            col = (jq * 2 + k) * 128
                            nc.tensor.matmul(
                                pvs[k][:],
                                vt_in[e][:, col : col + 128],
                                wv_t[e][:],
                                start=(e == 0),
                                stop=(e == 7),
                            )
                    for k in range(2):
                        jt = jh * 8 + jq * 2 + k
                        vv = v_tiles[jt].rearrange("p (h c) -> p h c", h=HL)
                        nc.vector.tensor_copy(
                            out=vv[:, :, 0:64],
                            in_=pvs[k][:].rearrange("p (h c) -> p h c", h=HL),
                        )

            # ---- late-loaded constants (not needed until out-proj / LN) ----
            wo_t = []
            for ot in range(4):
                w = wop.tile([128, E], F32R, tag="wo", name=f"wo_{ot}")
                nc.sync.dma_start(
                    out=w, in_=woT[ot * 128 : (ot + 1) * 128, :].bitcast(F32R)
                )
                wo_t.append(w)
            gamma_b = constp.tile([128, E], F32)
            nc.sync.dma_start(out=gamma_b, in_=vec3[0, :].partition_broadcast(128))
            beta_b = constp.tile([128, E], F32)
            nc.sync.dma_start(out=beta_b, in_=vec3[1, :].partition_broadcast(128))

            # ---------------- attention per local head ----------------
            ctxT = [
                ctxp.tile([128, SQ], F32R, tag="ctx", name=f"ctxT_{t}")
                for t in range(4)
            ]
            for h in range(HL):
                qt_tile = QTt[h // 2]
                kt_tile = KTt[h // 2]
                r0 = 64 * (h % 2)
                pcs = [
                    pc.tile([128, 512], F32, tag="ctx", name=f"pctx_{h}_{ih}")
                    for ih in range(2)
                ]
                for jt in range(16):
                    et = expp.tile([128, SQ], BF16, tag="exp", name=f"exp_{h}_{jt}")
                    sp = psc.tile([128, 1024], F32, tag="sc", name=f"sc_{h}_{jt}")
                    for ih in range(2):
                        nc.tensor.matmul(
                            sp[:, ih * 512 : (ih + 1) * 512],
                            kt_tile[r0 : r0 + 64, jt * 128 : (jt + 1) * 128],
                            qt_tile[r0 : r0 + 64, ih * 512 : (ih + 1) * 512],
                            start=True,
                            stop=True,
                        )
                    # one big exp over both i-halves (amortizes ACT overhead)
                    nc.scalar.activation(
                        out=et[:], in_=sp[:], func=AF.Exp, scale=0.125
                    )
                    for ih in range(2):
                        nc.tensor.matmul(
                            pcs[ih][0:65, :],
                            v_tiles[jt][:, h * 65 : (h + 1) * 65],
                            et[:, ih * 512 : (ih + 1) * 512],
                            start=(jt == 0),
                            stop=(jt == 15),
                        )
                # normalize: ctxT[d, i] = ctx~[d, i] / denom[i]
                for ih in range(2):
                    rec = rbp.tile([1, 512], F32R, tag="rec", name=f"rec_{h}_{ih}")
                    # f32r stores full fp32 bits; only PE reads are reduced
                    with nc.allow_low_precision(reason="f32r == fp32 storage"):
                        nc.vector.reciprocal(out=rec, in_=pcs[ih][64:65, :])
                    # replicate the reciprocal row down 64 partitions via a
                    # K=1 matmul (PSUM dst must start at partition 0)
                    pbt = pp.tile([128, 512], F32, tag="proj", name=f"pb_{h}_{ih}")
                    nc.tensor.matmul(
                        pbt[0:64, :], ones_r[:], rec[:], start=True, stop=True
                    )
                    rb = rbp.tile([64, 512], F32, tag="rb", name=f"rb_{h}_{ih}")
                    nc.vector.tensor_copy(out=rb, in_=pbt[0:64, :])
                    nc.vector.scalar_tensor_tensor(
                        out=ctxT[h // 2][r0 : r0 + 64, ih * 512 : (ih + 1) * 512],
                        in0=pcs[ih][0:64, :],
                        scalar=1.0,
                        in1=rb[:],
                        op0=OP.mult,
                        op1=OP.mult,
                    )

            # ------- pairwise ctx exchange (AllToAll) + out-proj + LN -------
            # Each core holds ctxT for its 512 head-dims over ALL 1024 rows.
            # The pair swaps row-halves of ctxT (1MB each way) so each core
            # gets the FULL 1024 head-dims for its own 512 rows, then runs the
            # complete out-projection + residual + LayerNorm locally with no
            # further communication. AllToAll chunk r = ctxT[:, 512r:512r+512]
            # (the block destined for rank r) -- identical program on both
            # ranks; output chunk r = rank r's head-dims for my rows.
            myctx = dramp.tile([2 * OS, OS], F32, tag="myctx")
            gath = dramp.tile([2 * OS, OS], F32, tag="gath")
            for ot in range(4):
                for r in range(2):
                    nc.sync.dma_start(
                        out=myctx[r * OS + ot * 128 : r * OS + (ot + 1) * 128, :],
                        in_=ctxT[ot][:, r * OS : (r + 1) * OS].bitcast(F32),
                    )
            nc.gpsimd.collective_compute(
                "AllToAll",
                OP.bypass,
                replica_groups=[[0, 1], [2, 3], [4, 5], [6, 7]],
                ins=[myctx[:]],
                outs=[gath[:]],
            )
            # full-ctx lhsT tiles for my 512 rows (recycle the V pool slots)
            fc = []
            for t in range(8):
                f_ = vsb.tile([128, OS], F32R, tag="v", name=f"fc_{t}")
                nc.sync.dma_start(
                    out=f_, in_=gath[t * 128 : (t + 1) * 128, :].bitcast(F32R)
                )
                fc.append(f_)

            for it in range(4):
                x = lnp.tile([128, E], F32, tag="x", name=f"x_{it}")
                for eh in range(2):
                    po = pp.tile([128, 512], F32, tag="proj", name=f"po_{it}_{eh}")
                    for t in range(8):
                        nc.tensor.matmul(
                            po[:],
                            fc[t][:, it * 128 : (it + 1) * 128],
                            wo_t[t][:, eh * 512 : (eh + 1) * 512],
                            start=(t == 0),
                            stop=(t == 7),
                        )
                    nc.scalar.copy(out=x[:, eh * 512 : (eh + 1) * 512], in_=po[:])
                r = actp.tile([128, E], F32, tag="act", name=f"res_{it}")
                nc.sync.dma_start(
                    out=r, in_=resid[it * 128 : (it + 1) * 128, :]
                )
                nc.vector.tensor_add(out=x, in0=x, in1=r)
                nc.vector.tensor_add(out=x, in0=x, in1=bo_b)
                st = lnp.tile([128, 2, 6], F32, tag="st", name=f"st_{it}")
                xg = x.rearrange("p (g d) -> p g d", g=2)
                for sg in range(2):
                    nc.vector.bn_stats(out=st[:, sg, :], in_=xg[:, sg, :])
                mv = lnp.tile([128, 2], F32, tag="mv", name=f"mv_{it}")
                nc.vector.bn_aggr(out=mv, in_=st)
                sd = lnp.tile([128, 1], F32, tag="sd", name=f"sd_{it}")
                nc.scalar.activation(
                    out=sd, in_=mv[:, 1:2], func=AF.Sqrt, bias=eps_t, scale=1.0
                )
                nc.vector.reciprocal(out=sd, in_=sd)
                nc.vector.tensor_scalar(
                    out=x,
                    in0=x,
                    scalar1=mv[:, 0:1],
                    scalar2=sd,
                    op0=OP.subtract,
                    op1=OP.mult,
                )
                y = lnp.tile([128, E], F32, tag="y", name=f"y_{it}")
                nc.vector.scalar_tensor_tensor(
                    out=y,
                    in0=x,
                    scalar=1.0,
                    in1=gamma_b,
                    op0=OP.mult,
                    op1=OP.mult,
                )
                nc.vector.tensor_add(out=y, in0=y, in1=beta_b)
                nc.sync.dma_start(
                    out=out[it * 128 : (it + 1) * 128, :], in_=y
                )

    nc.finalize()
    return nc


def build_in_maps(inputs):
    q = np.asarray(inputs["query"], dtype=np.float32)
    k = np.asarray(inputs["key"], dtype=np.float32)
    v = np.asarray(inputs["value"], dtype=np.float32)
    Wq = np.asarray(inputs["Wq"], dtype=np.float32)
    bq = np.asarray(inputs["bq"], dtype=np.float32)
    Wk = np.asarray(inputs["Wk"], dtype=np.float32)
    bk = np.asarray(inputs["bk"], dtype=np.float32)
    Wv = np.asarray(inputs["Wv"], dtype=np.float32)
    bv = np.asarray(inputs["bv"], dtype=np.float32)
    Wo = np.asarray(inputs["Wo"], dtype=np.float32)
    bo = np.asarray(inputs["bo"], dtype=np.float32)
    gamma = np.asarray(inputs["gamma"], dtype=np.float32)
    beta = np.asarray(inputs["beta"], dtype=np.float32)

    qT = [np.ascontiguousarray(q[b].T).astype(ml_dtypes.bfloat16) for b in range(B)]
    kT = [np.ascontiguousarray(k[b].T).astype(ml_dtypes.bfloat16) for b in range(B)]
    vT = [np.ascontiguousarray(v[b].T).astype(ml_dtypes.bfloat16) for b in range(B)]

    # bv folded into a host-side bias vector: out includes +bv @ Wo.T + bo.
    bo_eff = (bv @ Wo.T + bo).astype(np.float32)
    ones32 = np.ones((1, 64), dtype=np.float32)
    ones_bf = np.ones((1, 64), dtype=np.float32).astype(ml_dtypes.bfloat16)

    in_maps = []
    for c in range(N_CORES):
        b, g = divmod(c, 2)
        sl = slice(OS * g, OS * g + OS)
        in_maps.append(
            {
                "qT": qT[b],
                "kT": kT[b],
                "vT": vT[b],
                "wqT": np.ascontiguousarray(Wq[sl, :].T).astype(ml_dtypes.bfloat16),
                "wkT": np.ascontiguousarray(Wk[sl, :].T).astype(ml_dtypes.bfloat16),
                "wvT": np.ascontiguousarray(Wv[sl, :].T).astype(ml_dtypes.bfloat16),
                "woT": np.ascontiguousarray(Wo[:, sl].T),
                "bq4": np.ascontiguousarray(bq[sl].reshape(4, 128)),
                "bk4": np.ascontiguousarray(bk[sl].reshape(4, 128)),
                # rows owned after the 2-chunk RS: chunk k -> [512k+256g, +256)
                "resid": np.ascontiguousarray(
                    np.concatenate(
                        [
                            q[b, 256 * g : 256 * g + 256, :],
                            q[b, 512 + 256 * g : 512 + 256 * g + 256, :],
                        ]
                    )
                ),
                "vec3": np.ascontiguousarray(np.stack([gamma, beta])),
                "ones64": ones32,
                "ones_bf": ones_bf,
            }
        )
    return in_maps


def kernel(**inputs):
    global _NC_CACHE, LAST_RESULTS
    if _NC_CACHE is None:
        _NC_CACHE = _build_nc()
    nc = _NC_CACHE

    in_maps = build_in_maps(inputs)

    res = run_bass_kernel_spmd(nc, in_maps, list(range(N_CORES)), trace=TRACE)
    LAST_RESULTS = res

    outp = np.empty((B, SQ, E), dtype=np.float32)
    for c in range(N_CORES):
        b, g = divmod(c, 2)
        outp[b, OS * g : OS * g + OS, :] = res.results[c]["out"]
    return outp


# revision 52
# speedup vs baseline: 1.0206x; 1.0206x over previous
"""Multi-head cross-modal attention + residual + LayerNorm on 8 TRN2 cores.

Reference computation (per batch b):
  Q = query @ Wq.T + bq ; K = key @ Wk.T + bk ; V = value @ Wv.T + bv
  attn = softmax(Q K^T / sqrt(D)) per head
  out  = (attn V) @ Wo.T + bo
  y    = LayerNorm(out + query) * gamma + beta

Sharding: 2-D over (batch=4) x (head-group=2). Core c owns batch c//2 and
heads [8*(c%2), 8*(c%2)+8) i.e. a 512-wide slice of the embedding dim for
Q/K/V/ctx. The out-projection over the 512-slice yields partial sums that a
pairwise ReduceScatter (groups [0,1],[2,3],[4,5],[6,7]) combines; each core
then applies residual+LayerNorm to its 512 rows of the sequence and the host
concatenates the 8 [512,1024] results.

Layout strategy: host pre-transposes activations and weights so every matmul
operand already has its contraction dim on SBUF partitions => zero on-device
transposes. Scores are computed transposed (scoresT[j,i]); a ones-column
appended to V makes the softmax denominator fall out of the ctx matmul as
PSUM row 64. Softmax skips the max-subtraction (scores here are ~N(0,1);
max |score| << 80 so fp32 exp cannot overflow).

Precision: activations/weights stream in bf16 (fp32 accumulate in PSUM);
scores run as float32r (TF32-like, full PE rate at N>=256); the softmax
weights, V, and the reduce-scattered partials are bf16; LayerNorm runs in
fp32. Measured end-to-end max error vs the fp32 reference: ~2.7e-4 of the
output absmax.

Schedule: V/Q projections and the first K o-tile run up front; the remaining
K projection is software-pipelined one o-tile ahead inside the (ACT-bound)
attention head loops. The exp over both query halves is a single ScalarE
instruction per (head, j-tile) to amortize ACT instruction overhead.
"""

import sys

if "/opt/trn_rl_repo" not in sys.path:
    sys.path.insert(0, "/opt/trn_rl_repo")

import ml_dtypes
import numpy as np

import concourse.bass as bass  # noqa: F401  (registers types)
import concourse.mybir as mybir
import concourse.tile as tile
from concourse import bacc
from concourse.bass_utils import run_bass_kernel_spmd

F32 = mybir.dt.float32
F32R = mybir.dt.float32r
BF16 = mybir.dt.bfloat16
AF = mybir.ActivationFunctionType
OP = mybir.AluOpType

B, SQ, SK, E, H, D = 4, 1024, 2048, 1024, 16, 64
N_CORES = 8
OS = 512          # per-core slice of the embedding dim (8 heads x 64)
HL = 8            # local heads per core
ROWS = SQ // 2    # sequence rows each core owns after ReduceScatter
EPS = 1e-5

# module-level knobs used by test.py (harness ignores them)
TRACE = False
LAST_RESULTS = None

_NC_CACHE = None


def _build_nc():
    nc = bacc.Bacc(None, target_bir_lowering=False)

    qT = nc.dram_tensor("qT", [E, SQ], BF16, kind="ExternalInput")
    kT = nc.dram_tensor("kT", [E, SK], BF16, kind="ExternalInput")
    vT = nc.dram_tensor("vT", [E, SK], BF16, kind="ExternalInput")
    wqT = nc.dram_tensor("wqT", [E, OS], BF16, kind="ExternalInput")
    wkT = nc.dram_tensor("wkT", [E, OS], BF16, kind="ExternalInput")
    wvT = nc.dram_tensor("wvT", [E, OS], BF16, kind="ExternalInput")
    woT = nc.dram_tensor("woT", [OS, E], F32, kind="ExternalInput")
    bq4 = nc.dram_tensor("bq4", [4, 128], F32, kind="ExternalInput")
    bk4 = nc.dram_tensor("bk4", [4, 128], F32, kind="ExternalInput")
    resid = nc.dram_tensor("resid", [ROWS, E], F32, kind="ExternalInput")
    vec3 = nc.dram_tensor("vec3", [2, E], F32, kind="ExternalInput")
    ones64 = nc.dram_tensor("ones64", [1, 64], F32, kind="ExternalInput")
    ones_bf = nc.dram_tensor("ones_bf", [1, 64], BF16, kind="ExternalInput")
    out = nc.dram_tensor("out", [ROWS, E], F32, kind="ExternalOutput")

    from contextlib import ExitStack

    with ExitStack() as ctx:
        tc = ctx.enter_context(tile.TileContext(nc))
        constp = ctx.enter_context(tc.tile_pool(name="consts", bufs=1))
        wp = ctx.enter_context(tc.tile_pool(name="wp", bufs=9))
        wop = ctx.enter_context(tc.tile_pool(name="wop", bufs=4))
        actp = ctx.enter_context(tc.tile_pool(name="actp", bufs=8))
        qtp = ctx.enter_context(tc.tile_pool(name="qtp", bufs=4))
        ktp = ctx.enter_context(tc.tile_pool(name="ktp", bufs=4))
        vsb = ctx.enter_context(tc.tile_pool(name="vsb", bufs=16))
        expp = ctx.enter_context(tc.tile_pool(name="expp", bufs=4))
        ctxp = ctx.enter_context(tc.tile_pool(name="ctxp", bufs=4))
        stp = ctx.enter_context(tc.tile_pool(name="stp", bufs=2))
        lnp = ctx.enter_context(tc.tile_pool(name="lnp", bufs=2))
        rbp = ctx.enter_context(tc.tile_pool(name="rbp", bufs=2))
        pp = ctx.enter_context(tc.tile_pool(name="pp", bufs=2, space="PSUM"))
        psc = ctx.enter_context(tc.tile_pool(name="psc", bufs=2, space="PSUM"))
        pc = ctx.enter_context(tc.tile_pool(name="pc", bufs=2, space="PSUM"))
        dramp = ctx.enter_context(tc.tile_pool(name="dramp", bufs=1, space="DRAM"))
        if True:
            # ---------------- small constants (cheap DMAs) ----------------
            eps_t = constp.tile([128, 1], F32)
            nc.vector.memset(eps_t, EPS)
            ones_r = constp.tile([1, 64], F32R)
            nc.sync.dma_start(out=ones_r, in_=ones64[:].bitcast(F32R))
            bq_sb = constp.tile([128, 4], F32)
            bk_sb = constp.tile([128, 4], F32)
            for t in range(4):
                nc.sync.dma_start(
                    out=bq_sb[:, t : t + 1],
                    in_=bq4[t : t + 1, :].rearrange("a b -> b a"),
                )
                nc.sync.dma_start(
                    out=bk_sb[:, t : t + 1],
                    in_=bk4[t : t + 1, :].rearrange("a b -> b a"),
                )

            # ------------ small constants (after the big V loads) -----------
            ones_r = constp.tile([1, 64], F32R)
            nc.sync.dma_start(out=ones_r, in_=ones64[:].bitcast(F32R))
            bq_sb = constp.tile([128, 4], F32)
            bk_sb = constp.tile([128, 4], F32)
            for t in range(4):
                nc.sync.dma_start(
                    out=bq_sb[:, t : t + 1],
                    in_=bq4[t : t + 1, :].rearrange("a b -> b a"),
                )
                nc.sync.dma_start(
                    out=bk_sb[:, t : t + 1],
                    in_=bk4[t : t + 1, :].rearrange("a b -> b a"),
                )

            # ---------------- Q projection: QT[o, i] ----------------
            qt_in = []
            wq_t = []
            for e in range(8):
                w = wp.tile([128, OS], BF16, tag="w", name=f"wq_{e}")
                nc.sync.dma_start(
                    out=w, in_=wqT[e * 128 : (e + 1) * 128, :]
                )
                wq_t.append(w)
                a = actp.tile([128, SQ], BF16, tag="act", name=f"qin_{e}")
                nc.sync.dma_start(
                    out=a, in_=qT[e * 128 : (e + 1) * 128, :]
                )
                qt_in.append(a)
            QTt = [
                qtp.tile([128, SQ], BF16, tag="qt", name=f"QT_{ot}")
                for ot in range(4)
            ]
            for ot in range(4):
                for ih in range(2):
                    p = pp.tile([128, 512], F32, tag="proj", name=f"pq_{ot}_{ih}")
                    for e in range(8):
                        nc.tensor.matmul(
                            p[:],
                            wq_t[e][:, ot * 128 : (ot + 1) * 128],
                            qt_in[e][:, ih * 512 : (ih + 1) * 512],
                            start=(e == 0),
                            stop=(e == 7),
                        )
                    nc.vector.tensor_scalar_add(
                        out=QTt[ot][:, ih * 512 : (ih + 1) * 512],
                        in0=p[:],
                        scalar1=bq_sb[:, ot : ot + 1],
                    )

            # ---------------- K projection: KT[o, j] ----------------
            wk_t = []
            KTt = [
                ktp.tile([128, SK], BF16, tag="kt", name=f"KT_{ot}")
                for ot in range(4)
            ]
            for jh in range(2):
                kt_in = []
                for e in range(8):
                    if jh == 0:
                        w = wp.tile([128, OS], BF16, tag="w", name=f"wk_{e}")
                        nc.sync.dma_start(
                            out=w,
                            in_=wkT[e * 128 : (e + 1) * 128, :],
                        )
                        wk_t.append(w)
                    a = actp.tile([128, 1024], BF16, tag="act", name=f"kin_{jh}_{e}")
                    nc.sync.dma_start(
                        out=a,
                        in_=kT[
                            e * 128 : (e + 1) * 128, jh * 1024 : (jh + 1) * 1024
                        ],
                    )
                    kt_in.append(a)
                for ot in range(4):
                    for jc in range(2):
                        p = pp.tile(
                            [128, 512], F32, tag="proj", name=f"pk_{jh}_{ot}_{jc}"
                        )
                        for e in range(8):
                            nc.tensor.matmul(
                                p[:],
                                wk_t[e][:, ot * 128 : (ot + 1) * 128],
                                kt_in[e][:, jc * 512 : (jc + 1) * 512],
                                start=(e == 0),
                                stop=(e == 7),
                            )
                        off = jh * 1024 + jc * 512
                        nc.vector.tensor_scalar_add(
                            out=KTt[ot][:, off : off + 512],
                            in0=p[:],
                            scalar1=bk_sb[:, ot : ot + 1],
                        )

            # ---------------- V projection: V[j, (h,65)] with ones col ----
            wv_t = []
            v_tiles = [
                vsb.tile([128, HL * 65], BF16, tag="v", name=f"V_{jt}")
                for jt in range(16)
            ]
            for jh in range(2):
                vt_in = []
                for e in range(8):
                    if jh == 0:
                        w = wp.tile([128, OS], BF16, tag="w", name=f"wv_{e}")
                        nc.sync.dma_start(
                            out=w,
                            in_=wvT[e * 128 : (e + 1) * 128, :],
                        )
                        wv_t.append(w)
                    a = actp.tile([128, 1024], BF16, tag="act", name=f"vin_{jh}_{e}")
                    nc.sync.dma_start(
                        out=a,
                        in_=vT[
                            e * 128 : (e + 1) * 128, jh * 1024 : (jh + 1) * 1024
                        ],
                    )
                    vt_in.append(a)
                for jq4 in range(8):
                    jt = jh * 8 + jq4
                    vv = v_tiles[jt].rearrange("p (h c) -> p h c", h=HL)
                    nc.vector.memset(vv[:, :, 64:65], 1.0)
                for jq in range(4):
                    pvs = [
                        pp.tile([128, 512], F32, tag="proj", name=f"pv_{jh}_{jq}_{k}")
                        for k in range(2)
                    ]
                    for e in range(8):
                        for k in range(2):
                            col = (jq * 2 + k) * 128
                            nc.tensor.matmul(
                                pvs[k][:],
                                vt_in[e][:, col : col + 128],
                                wv_t[e][:],
                                start=(e == 0),
                                stop=(e == 7),
                            )
                    for k in range(2):
                        jt = jh * 8 + jq * 2 + k
                        vv = v_tiles[jt].rearrange("p (h c) -> p h c", h=HL)
                        nc.vector.tensor_copy(
                            out=vv[:, :, 0:64],
                            in_=pvs[k][:].rearrange("p (h c) -> p h c", h=HL),
                        )

            # ---- late-loaded constants (not needed until out-proj / LN) ----
            wo_t = []
            for ot in range(4):
                w = wop.tile([128, E], F32R, tag="wo", name=f"wo_{ot}")
                nc.sync.dma_start(
                    out=w, in_=woT[ot * 128 : (ot + 1) * 128, :].bitcast(F32R)
                )
                wo_t.append(w)
            gamma_b = constp.tile([128, E], F32)
            nc.sync.dma_start(out=gamma_b, in_=vec3[0, :].partition_broadcast(128))
            beta_b = constp.tile([128, E], F32)
            nc.sync.dma_start(out=beta_b, in_=vec3[1, :].partition_broadcast(128))

            # ---------------- attention per local head ----------------
            ctxT = [
                ctxp.tile([128, SQ], F32R, tag="ctx", name=f"ctxT_{t}")
                for t in range(4)
            ]
            for h in range(HL):
                qt_tile = QTt[h // 2]
                kt_tile = KTt[h // 2]
                r0 = 64 * (h % 2)
                pcs = [
                    pc.tile([128, 512], F32, tag="ctx", name=f"pctx_{h}_{ih}")
                    for ih in range(2)
                ]
                for jt in range(16):
                    et = expp.tile([128, SQ], BF16, tag="exp", name=f"exp_{h}_{jt}")
                    sp = psc.tile([128, 1024], F32, tag="sc", name=f"sc_{h}_{jt}")
                    for ih in range(2):
                        nc.tensor.matmul(
                            sp[:, ih * 512 : (ih + 1) * 512],
                            kt_tile[r0 : r0 + 64, jt * 128 : (jt + 1) * 128],
                            qt_tile[r0 : r0 + 64, ih * 512 : (ih + 1) * 512],
                            start=True,
                            stop=True,
                        )
                    # one big exp over both i-halves (amortizes ACT overhead)
                    nc.scalar.activation(
                        out=et[:], in_=sp[:], func=AF.Exp, scale=0.125
                    )
                    for ih in range(2):
                        nc.tensor.matmul(
                            pcs[ih][0:65, :],
                            v_tiles[jt][:, h * 65 : (h + 1) * 65],
                            et[:, ih * 512 : (ih + 1) * 512],
                            start=(jt == 0),
                            stop=(jt == 15),
                        )
                # normalize: ctxT[d, i] = ctx~[d, i] / denom[i]
                for ih in range(2):
                    rec = rbp.tile([1, 512], F32R, tag="rec", name=f"rec_{h}_{ih}")
                    # f32r stores full fp32 bits; only PE reads are reduced
                    with nc.allow_low_precision(reason="f32r == fp32 storage"):
                        nc.vector.reciprocal(out=rec, in_=pcs[ih][64:65, :])
                    # replicate the reciprocal row down 64 partitions via a
                    # K=1 matmul (PSUM dst must start at partition 0)
                    pbt = pp.tile([128, 512], F32, tag="proj", name=f"pb_{h}_{ih}")
                    nc.tensor.matmul(
                        pbt[0:64, :], ones_r[:], rec[:], start=True, stop=True
                    )
                    rb = rbp.tile([64, 512], F32, tag="rb", name=f"rb_{h}_{ih}")
                    nc.vector.tensor_copy(out=rb, in_=pbt[0:64, :])
                    nc.vector.scalar_tensor_tensor(
                        out=ctxT[h // 2][r0 : r0 + 64, ih * 512 : (ih + 1) * 512],
                        in0=pcs[ih][0:64, :],
                        scalar=1.0,
                        in1=rb[:],
                        op0=OP.mult,
                        op1=OP.mult,
                    )

            # ------- pairwise ctx exchange (AllToAll) + out-proj + LN -------
            # Each core holds ctxT for its 512 head-dims over ALL 1024 rows.
            # The pair swaps row-halves of ctxT (1MB each way) so each core
            # gets the FULL 1024 head-dims for its own 512 rows, then runs the
            # complete out-projection + residual + LayerNorm locally with no
            # further communication. AllToAll chunk r = ctxT[:, 512r:512r+512]
            # (the block destined for rank r) -- identical program on both
            # ranks; output chunk r = rank r's head-dims for my rows.
            myctx = dramp.tile([2 * OS, OS], F32, tag="myctx")
            gath = dramp.tile([2 * OS, OS], F32, tag="gath")
            for ot in range(4):
                for r in range(2):
                    nc.sync.dma_start(
                        out=myctx[r * OS + ot * 128 : r * OS + (ot + 1) * 128, :],
                        in_=ctxT[ot][:, r * OS : (r + 1) * OS].bitcast(F32),
                    )
            nc.gpsimd.collective_compute(
                "AllToAll",
                OP.bypass,
                replica_groups=[[0, 1], [2, 3], [4, 5], [6, 7]],
                ins=[myctx[:]],
                outs=[gath[:]],
            )
            # full-ctx lhsT tiles for my 512 rows (recycle the V pool slots)
            fc = []
            for t in range(8):
                f_ = vsb.tile([128, OS], F32R, tag="v", name=f"fc_{t}")
                nc.sync.dma_start(
                    out=f_, in_=gath[t * 128 : (t + 1) * 128, :].bitcast(F32R)
                )
                fc.append(f_)

            for it in range(4):
                x = lnp.tile([128, E], F32, tag="x", name=f"x_{it}")
                for eh in range(2):
                    po = pp.tile([128, 512], F32, tag="proj", name=f"po_{it}_{eh}")
                    for t in range(8):
                        nc.tensor.matmul(
                            po[:],
                            fc[t][:, it * 128 : (it + 1) * 128],
                            wo_t[t][:, eh * 512 : (eh + 1) * 512],
                            start=(t == 0),
                            stop=(t == 7),
                        )
                    nc.scalar.copy(out=x[:, eh * 512 : (eh + 1) * 512], in_=po[:])
                r = actp.tile([128, E], F32, tag="act", name=f"res_{it}")
                nc.sync.dma_start(
                    out=r, in_=resid[it * 128 : (it + 1) * 128, :]
                )
                nc.vector.tensor_add(out=x, in0=x, in1=r)
                nc.vector.tensor_add(out=x, in0=x, in1=bo_b)
                st = lnp.tile([128, 2, 6], F32, tag="st", name=f"st_{it}")
                xg = x.rearrange("p (g d) -> p g d", g=2)
                for sg in range(2):
                    nc.vector.bn_stats(out=st[:, sg, :], in_=xg[:, sg, :])
                mv = lnp.tile([128, 2], F32, tag="mv", name=f"mv_{it}")
                nc.vector.bn_aggr(out=mv, in_=st)
                sd = lnp.tile([128, 1], F32, tag="sd", name=f"sd_{it}")
                nc.scalar.activation(
                    out=sd, in_=mv[:, 1:2], func=AF.Sqrt, bias=eps_t, scale=1.0
                )
                nc.vector.reciprocal(out=sd, in_=sd)
                nc.vector.tensor_scalar(
                    out=x,
                    in0=x,
                    scalar1=mv[:, 0:1],
                    scalar2=sd,
                    op0=OP.subtract,
                    op1=OP.mult,
                )
                y = lnp.tile([128, E], F32, tag="y", name=f"y_{it}")
                nc.vector.scalar_tensor_tensor(
                    out=y,
                    in0=x,
                    scalar=1.0,
                    in1=gamma_b,
                    op0=OP.mult,
                    op1=OP.mult,
                )
                nc.vector.tensor_add(out=y, in0=y, in1=beta_b)
                nc.sync.dma_start(
                    out=out[it * 128 : (it + 1) * 128, :], in_=y
                )

    nc.finalize()
    return nc


def build_in_maps(inputs):
    q = np.asarray(inputs["query"], dtype=np.float32)
    k = np.asarray(inputs["key"], dtype=np.float32)
    v = np.asarray(inputs["value"], dtype=np.float32)
    Wq = np.asarray(inputs["Wq"], dtype=np.float32)
    bq = np.asarray(inputs["bq"], dtype=np.float32)
    Wk = np.asarray(inputs["Wk"], dtype=np.float32)
    bk = np.asarray(inputs["bk"], dtype=np.float32)
    Wv = np.asarray(inputs["Wv"], dtype=np.float32)
    bv = np.asarray(inputs["bv"], dtype=np.float32)
    Wo = np.asarray(inputs["Wo"], dtype=np.float32)
    bo = np.asarray(inputs["bo"], dtype=np.float32)
    gamma = np.asarray(inputs["gamma"], dtype=np.float32)
    beta = np.asarray(inputs["beta"], dtype=np.float32)

    qT = [np.ascontiguousarray(q[b].T).astype(ml_dtypes.bfloat16) for b in range(B)]
    kT = [np.ascontiguousarray(k[b].T).astype(ml_dtypes.bfloat16) for b in range(B)]
    vT = [np.ascontiguousarray(v[b].T).astype(ml_dtypes.bfloat16) for b in range(B)]

    # bv folded into a host-side bias vector: out includes +bv @ Wo.T + bo.
    bo_eff = (bv @ Wo.T + bo).astype(np.float32)
    ones32 = np.ones((1, 64), dtype=np.float32)
    ones_bf = np.ones((1, 64), dtype=np.float32).astype(ml_dtypes.bfloat16)

    in_maps = []
    for c in range(N_CORES):
        b, g = divmod(c, 2)
        sl = slice(OS * g, OS * g + OS)
        in_maps.append(
            {
                "qT": qT[b],
                "kT": kT[b],
                "vT": vT[b],
                "wqT": np.ascontiguousarray(Wq[sl, :].T).astype(ml_dtypes.bfloat16),
                "wkT": np.ascontiguousarray(Wk[sl, :].T).astype(ml_dtypes.bfloat16),
                "wvT": np.ascontiguousarray(Wv[sl, :].T).astype(ml_dtypes.bfloat16),
                "woT": np.ascontiguousarray(Wo[:, sl].T),
                "bq4": np.ascontiguousarray(bq[sl].reshape(4, 128)),
                "bk4": np.ascontiguousarray(bk[sl].reshape(4, 128)),
                # rows owned after the 2-chunk RS: chunk k -> [512k+256g, +256)
                "resid": np.ascontiguousarray(
                    np.concatenate(
                        [
                            q[b, 256 * g : 256 * g + 256, :],
                            q[b, 512 + 256 * g : 512 + 256 * g + 256, :],
                        ]
                    )
                ),
                "vec3": np.ascontiguousarray(np.stack([gamma, beta])),
                "ones64": ones32,
                "ones_bf": ones_bf,
            }
        )
    return in_maps


def kernel(**inputs):
    global _NC_CACHE, LAST_RESULTS
    if _NC_CACHE is None:
        _NC_CACHE = _build_nc()
    nc = _NC_CACHE

    in_maps = build_in_maps(inputs)

    res = run_bass_kernel_spmd(nc, in_maps, list(range(N_CORES)), trace=TRACE)
    LAST_RESULTS = res

    outp = np.empty((B, SQ, E), dtype=np.float32)
    for c in range(N_CORES):
        b, g = divmod(c, 2)
        outp[b, OS * g : OS * g + OS, :] = res.results[c]["out"]
    return outp
